# revision 14
# baseline (speedup 1.0000x reference)
"""Causal self-attention (QK-RMSNorm + RoPE) Trainium2 kernel, v2.

Sharding (Megatron-style, per the TP-over-heads hint):
  8 cores = 2 (batch) x 4 (head groups of 4 heads).
  Each core computes qkv/attention for its 4 heads on its batch and a partial
  projection output (bf16); the host sums the 4 partials per batch and
  transposes (the device emits the output feature-major).

Per-core pipeline (all matmuls bf16, fp32 PSUM accumulation):
  phase 1 (per token tile): DMA the pre-transposed x^T column block (built on
    the host), run v/q/k matmul groups back-to-back; for q/k: RMS-norm stats
    via DVE square+segmented-reduce (bf16), rope on the raw activations with
    the 1/rms scale applied once after (rope is linear), PE head-transpose to
    [d, t] layout.
  phase 2 (per q block j, head): scores^T = k^T.T @ q^T with causal tile
    skipping and column-narrowed diagonal tiles, exp on ACT, bf16 pair
    accumulation on DVE, denominator reduce+broadcast via one bf16
    ones[128x128] matmul, y^T = v.T @ p^T (also narrowed); projection for
    block j-1 interleaved between heads of block j.
"""

import math
from contextlib import ExitStack

import numpy as np
import ml_dtypes

import concourse.bass as bass
import concourse.mybir as mybir
import concourse.tile as tile
from concourse import bacc

F32 = mybir.dt.float32
BF16 = mybir.dt.bfloat16
AF = mybir.ActivationFunctionType
AX = mybir.AxisListType
ALU = mybir.AluOpType

# Problem constants (hardcoded; kernel.py must be self-contained)
B, T, C, H, HD = 2, 2048, 2048, 16, 128
N_CORES = 8
DP = 2                 # data-parallel ways (batch)
TPW = N_CORES // DP    # tensor-parallel ways (head groups)
HG = H // TPW          # heads per core
EPS = 1e-6


def build_nc(T_=T, C_=C, HG_=HG, hd=HD, TQ=512):
    NT = T_ // 128       # token tiles
    NCt = C_ // 128      # contraction tiles for qkv
    NJ = T_ // TQ        # query-block tiles
    NO = C_ // 128       # output feature tiles
    R = TQ // 128        # 128-wide k tiles per query block (diagonal span)
    F1 = HG_ * hd        # width of one of q/k/v chunks on this core
    HB = hd // 2
    sm_scale = 1.0 / math.sqrt(hd)

    nc = bacc.Bacc(None, target_bir_lowering=False)
    xT = nc.dram_tensor("xT", [C_, T_], BF16, kind="ExternalInput")
    wqkvT = nc.dram_tensor("wqkvT", [C_, 3 * F1], BF16, kind="ExternalInput")
    wprojT = nc.dram_tensor("wprojT", [F1, C_], BF16, kind="ExternalInput")
    rope_q = nc.dram_tensor("rope_q", [T_, 4 * HB], BF16, kind="ExternalInput")
    rope_k = nc.dram_tensor("rope_k", [T_, 4 * HB], BF16, kind="ExternalInput")
    masks_d = nc.dram_tensor("masks", [R * 128, TQ], BF16, kind="ExternalInput")
    ident_d = nc.dram_tensor("ident", [128, 128], BF16, kind="ExternalInput")
    outT = nc.dram_tensor("outT", [C_, T_], BF16, kind="ExternalOutput")

    with tile.TileContext(nc) as tc, ExitStack() as big:
        persist = big.enter_context(tc.tile_pool(name="persist", bufs=1))

        v_all = persist.tile([128, NT, F1], BF16, tag="v_all")
        qkT = persist.tile([128, 2, HG_, T_], BF16, tag="qkT")
        ident = persist.tile([128, 128], BF16, tag="ident")
        ones128 = persist.tile([128, 128], BF16, tag="ones128")
        nc.vector.memset(ones128, 1.0)
        eps_t = persist.tile([128, 1], F32, tag="eps")
        nc.vector.memset(eps_t, EPS)

        # weight / rope / mask tiles (persistent)
        wt = persist.tile([128, 3, NCt, F1], BF16, tag="wt")
        rope_sb = persist.tile([128, 2, NT, 4 * HB], BF16, tag="rope")
        masks_t = persist.tile([128, R * TQ], BF16, tag="masks")
        wp = persist.tile([128, HG_, C_], BF16, tag="wp")

        # ---- staged input DMAs (three rings; order = priority) ----
        # gpsimd (SWDGE): x^T tiles + rope; scalar(ACT HWDGE): v weights;
        # sync (SP HWDGE): q/k weights, masks, wp, output stores.
        xt_pool = big.enter_context(tc.tile_pool(name="xt", bufs=3))
        xts = []

        def load_xt(i, ring):
            t_ = xt_pool.tile([128, NCt, 128], BF16, tag="xt")
            ring.dma_start(
                t_,
                xT[:, i * 128:(i + 1) * 128].rearrange("(n p) t -> p n t", p=128),
            )
            xts.append(t_)

        def load_w(fc, s, ring):
            ring.dma_start(
                wt[:, fc, 4 * s:4 * s + 4, :],
                wqkvT[:, fc * F1:(fc + 1) * F1]
                .rearrange("(n p) f -> p n f", p=128)[:, 4 * s:4 * s + 4, :],
            )

        # HWDGE rings start fast; SWDGE (gpsimd) has ~8us launch latency.
        # sync: xT0/xT1 + q weights + rope_k; scalar: v weights + rope_q;
        # vector: k weights (DVE idle until ~8us); gpsimd: remaining xT.
        load_xt(0, nc.sync)
        for s in range(4):
            load_w(2, s, nc.scalar)
        load_xt(1, nc.sync)
        for s in range(4):
            load_w(0, s, nc.sync)
        for s in range(4):
            load_w(1, s, nc.scalar)
        nc.sync.dma_start(
            rope_sb[:, 0], rope_q[:].rearrange("(n p) f -> p n f", p=128))
        nc.sync.dma_start(
            rope_sb[:, 1], rope_k[:].rearrange("(n p) f -> p n f", p=128))
        nc.scalar.dma_start(ident, ident_d[:])
        for r in range(R):
            nc.sync.dma_start(
                masks_t[:, r * TQ:(r + 1) * TQ], masks_d[r * 128:(r + 1) * 128, :]
            )
        for ci in range(HG_):
            nc.sync.dma_start(wp[:, ci, :], wprojT[ci * 128:(ci + 1) * 128, :])

        scr = big.enter_context(tc.tile_pool(name="scr", bufs=2))
        stat = big.enter_context(tc.tile_pool(name="stat", bufs=2))

        def rope_ap(qk, i, c_idx):
            bse = rope_sb[:, qk, i, c_idx * HB:(c_idx + 1) * HB]
            return bass.AP(
                tensor=bse.tensor,
                offset=bse.offset,
                ap=[list(bse.ap[0]), [0, HG_], list(bse.ap[-1])],
            )

        def bcast_ap(t2, n_inner):
            # [128, G] -> [128, (G), (0 x n_inner)]
            bse = t2[:, :]
            return bass.AP(
                tensor=bse.tensor,
                offset=bse.offset,
                ap=[list(bse.ap[0]), list(bse.ap[-1]), [0, n_inner]],
            )

        # ---------------- phase 1: unified qkv loop ----------------
        with ExitStack() as ph1ps:
            ps_pool = ph1ps.enter_context(
                tc.tile_pool(name="ps_qkv", bufs=5, space="PSUM"))
            ps_tr = ph1ps.enter_context(
                tc.tile_pool(name="ps_tr", bufs=2, space="PSUM"))

            for i in range(NT):
                if i + 2 < NT:
                    load_xt(i + 2, nc.gpsimd)
                xti = xts[i]
                for fc in (2, 0, 1):
                    ps = ps_pool.tile([128, F1], F32, tag="ps")
                    for ci in range(NCt):
                        nc.tensor.matmul(
                            ps, xti[:, ci, :], wt[:, fc, ci, :],
                            start=(ci == 0), stop=(ci == NCt - 1),
                        )
                    if fc == 2:
                        if i % 2 == 0:
                            nc.vector.tensor_copy(v_all[:, i, :], ps)
                        else:
                            nc.scalar.copy(v_all[:, i, :], ps)
                        continue
                    # ---- RMS norm stats + rope (on raw qb; rinv after) ----
                    qb = scr.tile([128, F1], BF16, tag="qb")
                    nc.scalar.copy(qb, ps)
                    q2 = scr.tile([128, F1], BF16, tag="q2")
                    nc.vector.tensor_mul(q2, qb, qb)
                    ssq = stat.tile([128, HG_], BF16, tag="ssq")
                    with nc.allow_low_precision(reason="rms stats averaged"):
                        nc.vector.tensor_reduce(
                            ssq, q2.rearrange("p (h d) -> p h d", h=HG_),
                            axis=AX.X, op=ALU.add,
                        )
                    sstd = stat.tile([128, HG_], BF16, tag="sstd")
                    nc.scalar.activation(
                        sstd, ssq, AF.Sqrt, bias=eps_t[:, 0:1], scale=1.0 / hd
                    )
                    rinv = stat.tile([128, HG_], BF16, tag="rinv")
                    with nc.allow_low_precision(reason="per-row scale"):
                        nc.vector.reciprocal(rinv, sstd)
                    qb3 = qb.rearrange("p (h two d) -> p h two d", h=HG_, two=2)
                    tm1 = scr.tile([128, HG_ * HB], BF16, tag="tm1")
                    tm2 = scr.tile([128, HG_ * HB], BF16, tag="tm2")
                    tm3 = scr.tile([128, HG_ * HB], BF16, tag="tm3")
                    tm4 = scr.tile([128, HG_ * HB], BF16, tag="tm4")
                    t13 = tm1.rearrange("p (h d) -> p h d", h=HG_)
                    t23 = tm2.rearrange("p (h d) -> p h d", h=HG_)
                    t33 = tm3.rearrange("p (h d) -> p h d", h=HG_)
                    t43 = tm4.rearrange("p (h d) -> p h d", h=HG_)
                    nc.vector.tensor_mul(t13, qb3[:, :, 0, :], rope_ap(fc, i, 0))
                    nc.gpsimd.tensor_mul(t23, qb3[:, :, 1, :], rope_ap(fc, i, 1))
                    nc.gpsimd.tensor_mul(t33, qb3[:, :, 1, :], rope_ap(fc, i, 2))
                    nc.vector.tensor_mul(t43, qb3[:, :, 0, :], rope_ap(fc, i, 3))
                    rq = scr.tile([128, F1], BF16, tag="rq")
                    rq3 = rq.rearrange("p (h two d) -> p h two d", h=HG_, two=2)
                    nc.vector.tensor_sub(rq3[:, :, 0, :], t13, t23)
                    nc.vector.tensor_add(rq3[:, :, 1, :], t33, t43)
                    rn = scr.tile([128, F1], BF16, tag="rn")
                    nc.vector.tensor_mul(
                        rn.rearrange("p (h d) -> p h d", h=HG_),
                        rq.rearrange("p (h d) -> p h d", h=HG_),
                        bcast_ap(rinv, hd),
                    )
                    pt = ps_tr.tile([128, F1], BF16, tag="tr")
                    for h in range(HG_):
                        nc.tensor.transpose(
                            pt[:, h * hd:(h + 1) * hd],
                            rn[:, h * hd:(h + 1) * hd], ident)
                    dst = qkT[:, fc, :, i * 128:(i + 1) * 128]
                    ptv = pt.rearrange("p (h t) -> p h t", h=HG_)
                    if i % 2 == 0:
                        nc.scalar.copy(dst, ptv)
                    else:
                        nc.vector.tensor_copy(dst, ptv)

        # -------- phase 2: attention with interleaved projection --------
        pP = big.enter_context(tc.tile_pool(name="pP", bufs=8))
        dP = big.enter_context(tc.tile_pool(name="dP", bufs=2))
        rbP = big.enter_context(tc.tile_pool(name="rbP", bufs=2))
        yP = big.enter_context(tc.tile_pool(name="yP", bufs=2))
        oP = big.enter_context(tc.tile_pool(name="oP", bufs=4))
        ps_s = big.enter_context(tc.tile_pool(name="ps_s", bufs=2, space="PSUM"))
        ps_yp = big.enter_context(tc.tile_pool(name="ps_y", bufs=2, space="PSUM"))
        ps_m = big.enter_context(tc.tile_pool(name="ps_m", bufs=2, space="PSUM"))

        yTs = {}

        def emit_proj(j, o_lo, o_hi):
            yTj = yTs[j]
            for o in range(o_lo, o_hi):
                pp = ps_m.tile([128, TQ], F32, tag="mix")
                for ci in range(HG_):
                    nc.tensor.matmul(
                        pp,
                        wp[:, ci, o * 128:(o + 1) * 128],
                        yTj[:, ci, :],
                        start=(ci == 0),
                        stop=(ci == HG_ - 1),
                    )
                ost = oP.tile([128, TQ], BF16, tag="ost")
                if o % 2 == 0:
                    nc.scalar.copy(ost, pp)
                else:
                    nc.vector.tensor_copy(ost, pp)
                nc.sync.dma_start(
                    outT[o * 128:(o + 1) * 128, j * TQ:(j + 1) * TQ], ost)

        def emit_den(j, h, dsum, daccB, yps):
            """Denominator merge + normalize for a finished head (emitted one
            head later so the accumulate chains drain off the PE path)."""
            yTj = yTs[j]
            den = ps_m.tile([128, TQ], F32, tag="mix")
            nc.tensor.matmul(den, ones128, dsum, start=True, stop=False)
            nc.tensor.matmul(den, ones128, daccB[:, :TQ], start=False, stop=False)
            nc.tensor.matmul(den, ones128, daccB[:, TQ:], start=False, stop=True)
            rb = rbP.tile([128, TQ], F32, tag="rb")
            nc.vector.reciprocal_approx_fast(rb, den)
            nc.vector.tensor_mul(yTj[:, h, :], yps, rb)

        pend = None  # (j, h, dsum, daccB, yps) awaiting denominator

        for j in range(NJ):
            yTj = yP.tile([128, HG_, TQ], BF16, tag="yT")
            yTs[j] = yTj
            nk = R * j + R          # valid 128-wide k tiles (causal)
            npairs = nk // 2
            for h in range(HG_):
                # two parallel denominator accumulators: even pairs on DVE,
                # odd pairs via SWDGE dma accumulate (keeps DVE off the
                # critical path); merged by the PE ones-matmul below.
                daccA = dP.tile([128, 2 * TQ], BF16, tag="daccA")
                daccB = dP.tile([128, 2 * TQ], BF16, tag="daccB")
                plist = []
                for kp in range(npairs):
                    s2 = ps_s.tile([128, 2 * TQ], F32, tag="s2")
                    r0 = 2 * kp - R * j
                    off0 = 128 * r0 if (j > 0 and r0 >= 1) else 0
                    for half in range(2):
                        k = 2 * kp + half
                        r = k - R * j
                        off = 128 * r if (j > 0 and r >= 1) else 0
                        nc.tensor.matmul(
                            s2[:, half * TQ + off:(half + 1) * TQ],
                            qkT[:, 1, h, k * 128:(k + 1) * 128],
                            qkT[:, 0, h, j * TQ + off:(j + 1) * TQ],
                            start=True,
                            stop=True,
                        )
                    p2 = pP.tile([128, 2 * TQ], BF16, tag="p2")
                    if off0:
                        nc.scalar.activation(
                            p2.rearrange("p (k t) -> p k t", k=2)[:, :, off0:],
                            s2.rearrange("p (k t) -> p k t", k=2)[:, :, off0:],
                            AF.Exp, scale=sm_scale)
                    else:
                        nc.scalar.activation(p2, s2, AF.Exp, scale=sm_scale)
                    if kp >= npairs - 2:  # the two diagonal-block pairs
                        r0m = 2 * kp - R * j
                        nc.vector.tensor_mul(
                            p2, p2, masks_t[:, r0m * TQ:(r0m + 2) * TQ]
                        )
                    if kp == 0:
                        nc.vector.tensor_copy(daccA, p2)
                    elif kp == 1:
                        nc.gpsimd.dma_start(daccB, p2)
                    elif kp % 2 == 0:
                        nc.vector.tensor_add(daccA, daccA, p2)
                    else:
                        nc.gpsimd.dma_start(daccB, p2, accum_op=ALU.add)
                    plist.append(p2)
                # fold the A-halves now so the add clears DVE before the next
                # head's backlog (consumed by emit_den one head later)
                dsum = dP.tile([128, TQ], BF16, tag="dsum")
                nc.vector.tensor_add(dsum, daccA[:, :TQ], daccA[:, TQ:])
                yps = ps_yp.tile([128, TQ], F32, tag="yps")
                for k in range(nk):
                    r = k - R * j
                    off = 128 * r if r >= 1 else 0
                    nc.tensor.matmul(
                        yps[:, off:],
                        v_all[:, k, h * hd:(h + 1) * hd],
                        plist[k // 2][:, (k % 2) * TQ + off:(k % 2 + 1) * TQ],
                        start=(k == 0),
                        stop=(k == nk - 1),
                    )
                if pend is not None:
                    emit_den(*pend)
                pend = (j, h, dsum, daccB, yps)
                # interleave previous block's projection between heads
                if j > 0 and h == 1:
                    emit_proj(j - 1, 0, NO // 2)
                elif j > 0 and h == 3:
                    emit_proj(j - 1, NO // 2, NO)
        emit_den(*pend)
        emit_proj(NJ - 1, 0, NO)

    nc.compile()
    return nc


def make_host_inputs(x, Wqkv, Wproj, q_norm_w, k_norm_w, rope_cos, rope_sin,
                     T_=T, C_=C, HG_=HG, hd=HD, TQ=512):
    """Build the 8 per-core input maps (sharding done on host)."""
    H_ = Wqkv.shape[0] // (3 * hd)
    tpw = H_ // HG_
    R = TQ // 128
    HB = hd // 2

    def rope_tables(w):
        # out1 = qb1*(cos*w1) - qb2*(sin*w2); out2 = qb2*(cos*w2) + qb1*(sin*w1)
        w1, w2 = w[:HB], w[HB:]
        A = rope_cos * w1[None, :]
        Bt = rope_sin * w2[None, :]
        Ct = rope_cos * w2[None, :]
        D = rope_sin * w1[None, :]
        return np.ascontiguousarray(
            np.concatenate([A, Bt, Ct, D], axis=1).astype(ml_dtypes.bfloat16)
        )

    rope_q_h = rope_tables(np.asarray(q_norm_w, dtype=np.float32))
    rope_k_h = rope_tables(np.asarray(k_norm_w, dtype=np.float32))

    # diagonal causal masks: pattern r: valid when tk + 128*r <= tq
    tk = np.arange(128)[:, None]
    tq = np.arange(TQ)[None, :]
    masks = np.concatenate(
        [(tk + 128 * r <= tq) for r in range(R)], axis=0
    ).astype(ml_dtypes.bfloat16)

    Wqkv = np.asarray(Wqkv, dtype=np.float32)
    Wproj = np.asarray(Wproj, dtype=np.float32)
    x = np.asarray(x, dtype=np.float32)
    xTs = [
        np.ascontiguousarray(x[b].T).astype(ml_dtypes.bfloat16)
        for b in range(x.shape[0])
    ]

    in_maps = []
    for core in range(N_CORES):
        b = core // tpw
        g = core % tpw
        rs = slice(g * HG_ * hd, (g + 1) * HG_ * hd)
        W_shard = np.concatenate(
            [Wqkv[0 * H_ * hd:][rs.start:rs.stop],
             Wqkv[1 * H_ * hd:][rs.start:rs.stop],
             Wqkv[2 * H_ * hd:][rs.start:rs.stop]], axis=0
        )  # [3*F1, C]
        in_maps.append({
            "ident": np.eye(128, dtype=ml_dtypes.bfloat16),
            "xT": xTs[b],
            "wqkvT": np.ascontiguousarray(W_shard.T).astype(ml_dtypes.bfloat16),
            "wprojT": np.ascontiguousarray(Wproj[:, rs].T).astype(ml_dtypes.bfloat16),
            "rope_q": rope_q_h,
            "rope_k": rope_k_h,
            "masks": masks,
        })
    return in_maps


_NC_CACHE = {}


def run_spmd(inputs, **run_kwargs):
    from concourse.bass_utils import run_bass_kernel_spmd

    x = np.asarray(inputs["x"])
    in_maps = make_host_inputs(
        x, inputs["Wqkv"], inputs["Wproj"], inputs["q_norm_w"],
        inputs["k_norm_w"], inputs["rope_cos"], inputs["rope_sin"],
    )
    if "nc" not in _NC_CACHE:
        _NC_CACHE["nc"] = build_nc()
    nc = _NC_CACHE["nc"]
    res = run_bass_kernel_spmd(nc, in_maps, core_ids=list(range(N_CORES)),
                               **run_kwargs)
    tpw = N_CORES // B
    out = np.zeros((B, T, C), dtype=np.float32)
    for core in range(N_CORES):
        b = core // tpw
        out[b] += res.results[core]["outT"].astype(np.float32).T
    return out, res


def kernel(**inputs):
    out, _ = run_spmd(inputs)
    return out


# revision 22
# speedup vs baseline: 1.0109x; 1.0109x over previous
"""Causal self-attention (QK-RMSNorm + RoPE) Trainium2 kernel, v2.

Sharding (Megatron-style, per the TP-over-heads hint):
  8 cores = 2 (batch) x 4 (head groups of 4 heads).
  Each core computes qkv/attention for its 4 heads on its batch and a partial
  projection output (bf16); the host sums the 4 partials per batch and
  transposes (the device emits the output feature-major).

Per-core pipeline (all matmuls bf16, fp32 PSUM accumulation):
  phase 1 (per token tile): DMA the pre-transposed x^T column block (built on
    the host), run v/q/k matmul groups back-to-back; for q/k: RMS-norm stats
    via DVE square+segmented-reduce (bf16), rope on the raw activations with
    the 1/rms scale applied once after (rope is linear), PE head-transpose to
    [d, t] layout.
  phase 2 (per q block j, head): scores^T = k^T.T @ q^T with causal tile
    skipping and column-narrowed diagonal tiles, exp on ACT, bf16 pair
    accumulation on DVE, denominator reduce+broadcast via one bf16
    ones[128x128] matmul, y^T = v.T @ p^T (also narrowed); projection for
    block j-1 interleaved between heads of block j.
"""

import math
from contextlib import ExitStack

import numpy as np
import ml_dtypes

import concourse.bass as bass
import concourse.mybir as mybir
import concourse.tile as tile
from concourse import bacc

F32 = mybir.dt.float32
BF16 = mybir.dt.bfloat16
AF = mybir.ActivationFunctionType
AX = mybir.AxisListType
ALU = mybir.AluOpType

# Problem constants (hardcoded; kernel.py must be self-contained)
B, T, C, H, HD = 2, 2048, 2048, 16, 128
N_CORES = 8
DP = 2                 # data-parallel ways (batch)
TPW = N_CORES // DP    # tensor-parallel ways (head groups)
HG = H // TPW          # heads per core
EPS = 1e-6


def build_nc(T_=T, C_=C, HG_=HG, hd=HD, TQ=512):
    NT = T_ // 128       # token tiles
    NCt = C_ // 128      # contraction tiles for qkv
    NJ = T_ // TQ        # query-block tiles
    NO = C_ // 128       # output feature tiles
    R = TQ // 128        # 128-wide k tiles per query block (diagonal span)
    F1 = HG_ * hd        # width of one of q/k/v chunks on this core
    HB = hd // 2
    sm_scale = 1.0 / math.sqrt(hd)

    nc = bacc.Bacc(None, target_bir_lowering=False)
    xT = nc.dram_tensor("xT", [C_, T_], BF16, kind="ExternalInput")
    wqkvT = nc.dram_tensor("wqkvT", [C_, 3 * F1], BF16, kind="ExternalInput")
    wprojT = nc.dram_tensor("wprojT", [F1, C_], BF16, kind="ExternalInput")
    rope_q = nc.dram_tensor("rope_q", [T_, 4 * HB], BF16, kind="ExternalInput")
    rope_k = nc.dram_tensor("rope_k", [T_, 4 * HB], BF16, kind="ExternalInput")
    masks_d = nc.dram_tensor("masks", [R * 128, TQ], BF16, kind="ExternalInput")
    ident_d = nc.dram_tensor("ident", [128, 128], BF16, kind="ExternalInput")
    outT = nc.dram_tensor("outT", [C_, T_], BF16, kind="ExternalOutput")

    with tile.TileContext(nc) as tc, ExitStack() as big:
        persist = big.enter_context(tc.tile_pool(name="persist", bufs=1))

        v_all = persist.tile([128, NT, F1], BF16, tag="v_all")
        qkT = persist.tile([128, 2, HG_, T_], BF16, tag="qkT")
        ident = persist.tile([128, 128], BF16, tag="ident")
        ones128 = persist.tile([128, 128], BF16, tag="ones128")
        nc.vector.memset(ones128, 1.0)
        eps_t = persist.tile([128, 1], F32, tag="eps")
        nc.vector.memset(eps_t, EPS)

        # weight / rope / mask tiles (persistent)
        wt = persist.tile([128, 3, NCt, F1], BF16, tag="wt")
        rope_sb = persist.tile([128, 2, NT, 4 * HB], BF16, tag="rope")
        masks_t = persist.tile([128, R * TQ], BF16, tag="masks")
        wp = persist.tile([128, HG_, C_], BF16, tag="wp")

        # ---- staged input DMAs (three rings; order = priority) ----
        # gpsimd (SWDGE): x^T tiles + rope; scalar(ACT HWDGE): v weights;
        # sync (SP HWDGE): q/k weights, masks, wp, output stores.
        xt_pool = big.enter_context(tc.tile_pool(name="xt", bufs=3))
        xts = []

        def load_xt(i, ring):
            t_ = xt_pool.tile([128, NCt, 128], BF16, tag="xt")
            ring.dma_start(
                t_,
                xT[:, i * 128:(i + 1) * 128].rearrange("(n p) t -> p n t", p=128),
            )
            xts.append(t_)

        def load_w(fc, s, ring):
            ring.dma_start(
                wt[:, fc, 4 * s:4 * s + 4, :],
                wqkvT[:, fc * F1:(fc + 1) * F1]
                .rearrange("(n p) f -> p n f", p=128)[:, 4 * s:4 * s + 4, :],
            )

        # HWDGE rings start fast; SWDGE (gpsimd) has ~8us launch latency.
        # sync: v/q weights + rope_q; scalar: xT0/xT1 + k weights + rope_k;
        # gpsimd: remaining xT tiles.
        for s in range(4):
            load_w(2, s, nc.sync)
        load_xt(0, nc.scalar)
        load_xt(1, nc.scalar)
        for s in range(4):
            load_w(0, s, nc.sync)
        for s in range(4):
            load_w(1, s, nc.scalar)
        nc.sync.dma_start(
            rope_sb[:, 0], rope_q[:].rearrange("(n p) f -> p n f", p=128))
        nc.scalar.dma_start(
            rope_sb[:, 1], rope_k[:].rearrange("(n p) f -> p n f", p=128))
        nc.scalar.dma_start(ident, ident_d[:])
        for r in range(R):
            nc.sync.dma_start(
                masks_t[:, r * TQ:(r + 1) * TQ], masks_d[r * 128:(r + 1) * 128, :]
            )
        for ci in range(HG_):
            nc.sync.dma_start(wp[:, ci, :], wprojT[ci * 128:(ci + 1) * 128, :])

        scr = big.enter_context(tc.tile_pool(name="scr", bufs=2))
        stat = big.enter_context(tc.tile_pool(name="stat", bufs=2))

        def rope_ap(qk, i, c_idx):
            bse = rope_sb[:, qk, i, c_idx * HB:(c_idx + 1) * HB]
            return bass.AP(
                tensor=bse.tensor,
                offset=bse.offset,
                ap=[list(bse.ap[0]), [0, HG_], list(bse.ap[-1])],
            )

        def bcast_ap(t2, n_inner):
            # [128, G] -> [128, (G), (0 x n_inner)]
            bse = t2[:, :]
            return bass.AP(
                tensor=bse.tensor,
                offset=bse.offset,
                ap=[list(bse.ap[0]), list(bse.ap[-1]), [0, n_inner]],
            )

        # ---------------- phase 1: unified qkv loop ----------------
        with ExitStack() as ph1ps:
            ps_pool = ph1ps.enter_context(
                tc.tile_pool(name="ps_qkv", bufs=5, space="PSUM"))
            ps_tr = ph1ps.enter_context(
                tc.tile_pool(name="ps_tr", bufs=2, space="PSUM"))

            pend_tr = []   # deferred (rn, fc, i) head-transposes

            def emit_tr(rn, fc, i):
                pt = ps_tr.tile([128, F1], BF16, tag="tr")
                for h in range(HG_):
                    nc.tensor.transpose(
                        pt[:, h * hd:(h + 1) * hd],
                        rn[:, h * hd:(h + 1) * hd], ident)
                dst = qkT[:, fc, :, i * 128:(i + 1) * 128]
                ptv = pt.rearrange("p (h t) -> p h t", h=HG_)
                if i % 2 == 0:
                    nc.scalar.copy(dst, ptv)
                else:
                    nc.vector.tensor_copy(dst, ptv)

            for i in range(NT):
                if i + 2 < NT:
                    load_xt(i + 2, nc.gpsimd)
                xti = xts[i]
                for fc in (2, 0, 1):
                    ps = ps_pool.tile([128, F1], F32, tag="ps")
                    for ci in range(NCt):
                        nc.tensor.matmul(
                            ps, xti[:, ci, :], wt[:, fc, ci, :],
                            start=(ci == 0), stop=(ci == NCt - 1),
                        )
                    # drain one deferred transpose group between matmul groups
                    # (skip the fc==1 slot so each group gets a full tile of
                    # slack for its rope chain)
                    if pend_tr and fc != 1:
                        emit_tr(*pend_tr.pop(0))
                    if fc == 2:
                        if i % 2 == 0:
                            nc.vector.tensor_copy(v_all[:, i, :], ps)
                        else:
                            nc.scalar.copy(v_all[:, i, :], ps)
                        continue
                    # ---- RMS norm stats + rope (on raw qb; rinv after) ----
                    qb = scr.tile([128, F1], BF16, tag="qb")
                    nc.scalar.copy(qb, ps)
                    q2 = scr.tile([128, F1], BF16, tag="q2")
                    nc.vector.tensor_mul(q2, qb, qb)
                    ssq = stat.tile([128, HG_], BF16, tag="ssq")
                    with nc.allow_low_precision(reason="rms stats averaged"):
                        nc.vector.tensor_reduce(
                            ssq, q2.rearrange("p (h d) -> p h d", h=HG_),
                            axis=AX.X, op=ALU.add,
                        )
                    sstd = stat.tile([128, HG_], BF16, tag="sstd")
                    nc.scalar.activation(
                        sstd, ssq, AF.Sqrt, bias=eps_t[:, 0:1], scale=1.0 / hd
                    )
                    rinv = stat.tile([128, HG_], BF16, tag="rinv")
                    with nc.allow_low_precision(reason="per-row scale"):
                        nc.vector.reciprocal(rinv, sstd)
                    qb3 = qb.rearrange("p (h two d) -> p h two d", h=HG_, two=2)
                    tm1 = scr.tile([128, HG_ * HB], BF16, tag="tm1")
                    tm2 = scr.tile([128, HG_ * HB], BF16, tag="tm2")
                    tm3 = scr.tile([128, HG_ * HB], BF16, tag="tm3")
                    tm4 = scr.tile([128, HG_ * HB], BF16, tag="tm4")
                    t13 = tm1.rearrange("p (h d) -> p h d", h=HG_)
                    t23 = tm2.rearrange("p (h d) -> p h d", h=HG_)
                    t33 = tm3.rearrange("p (h d) -> p h d", h=HG_)
                    t43 = tm4.rearrange("p (h d) -> p h d", h=HG_)
                    nc.vector.tensor_mul(t13, qb3[:, :, 0, :], rope_ap(fc, i, 0))
                    nc.gpsimd.tensor_mul(t23, qb3[:, :, 1, :], rope_ap(fc, i, 1))
                    nc.gpsimd.tensor_mul(t33, qb3[:, :, 1, :], rope_ap(fc, i, 2))
                    nc.vector.tensor_mul(t43, qb3[:, :, 0, :], rope_ap(fc, i, 3))
                    rq = scr.tile([128, F1], BF16, tag="rq")
                    rq3 = rq.rearrange("p (h two d) -> p h two d", h=HG_, two=2)
                    nc.vector.tensor_sub(rq3[:, :, 0, :], t13, t23)
                    nc.vector.tensor_add(rq3[:, :, 1, :], t33, t43)
                    rn = scr.tile([128, F1], BF16, tag="rn")
                    nc.vector.tensor_mul(
                        rn.rearrange("p (h d) -> p h d", h=HG_),
                        rq.rearrange("p (h d) -> p h d", h=HG_),
                        bcast_ap(rinv, hd),
                    )
                    pend_tr.append((rn, fc, i))
            while pend_tr:
                emit_tr(*pend_tr.pop(0))

        # -------- phase 2: attention with interleaved projection --------
        pP = big.enter_context(tc.tile_pool(name="pP", bufs=8))
        dP = big.enter_context(tc.tile_pool(name="dP", bufs=3))
        rbP = big.enter_context(tc.tile_pool(name="rbP", bufs=2))
        ybP = big.enter_context(tc.tile_pool(name="ybP", bufs=3))
        yP = big.enter_context(tc.tile_pool(name="yP", bufs=2))
        oP = big.enter_context(tc.tile_pool(name="oP", bufs=4))
        ps_s = big.enter_context(tc.tile_pool(name="ps_s", bufs=2, space="PSUM"))
        ps_yp = big.enter_context(tc.tile_pool(name="ps_y", bufs=2, space="PSUM"))
        ps_m = big.enter_context(tc.tile_pool(name="ps_m", bufs=2, space="PSUM"))

        yTs = {}

        def emit_proj(j, o_lo, o_hi):
            yTj = yTs[j]
            for o in range(o_lo, o_hi):
                pp = ps_m.tile([128, TQ], F32, tag="mix")
                for ci in range(HG_):
                    nc.tensor.matmul(
                        pp,
                        wp[:, ci, o * 128:(o + 1) * 128],
                        yTj[:, ci, :],
                        start=(ci == 0),
                        stop=(ci == HG_ - 1),
                    )
                ost = oP.tile([128, TQ], BF16, tag="ost")
                if o % 2 == 0:
                    nc.scalar.copy(ost, pp)
                else:
                    nc.vector.tensor_copy(ost, pp)
                nc.sync.dma_start(
                    outT[o * 128:(o + 1) * 128, j * TQ:(j + 1) * TQ], ost)

        def emit_den(j, h, dsum, daccB, yb):
            """Denominator merge + normalize for a finished head (emitted two
            heads later so the dma-accumulate chain drains off the PE path)."""
            yTj = yTs[j]
            den = ps_m.tile([128, TQ], F32, tag="mix")
            nc.tensor.matmul(den, ones128, dsum, start=True, stop=False)
            nc.tensor.matmul(den, ones128, daccB[:, :TQ], start=False, stop=False)
            nc.tensor.matmul(den, ones128, daccB[:, TQ:], start=False, stop=True)
            rb = rbP.tile([128, TQ], F32, tag="rb")
            nc.vector.reciprocal_approx_fast(rb, den)
            nc.vector.tensor_mul(yTj[:, h, :], yb, rb)

        pend = []  # [(j, h, dsum, daccB, yb)] awaiting denominator (depth 2)

        for j in range(NJ):
            yTj = yP.tile([128, HG_, TQ], BF16, tag="yT")
            yTs[j] = yTj
            nk = R * j + R          # valid 128-wide k tiles (causal)
            npairs = nk // 2
            for h in range(HG_):
                # two parallel denominator accumulators: even pairs on DVE,
                # odd pairs via SWDGE dma accumulate (keeps DVE off the
                # critical path); merged by the PE ones-matmul below.
                daccA = dP.tile([128, 2 * TQ], BF16, tag="daccA")
                daccB = dP.tile([128, 2 * TQ], BF16, tag="daccB")
                plist = []
                for kp in range(npairs):
                    s2 = ps_s.tile([128, 2 * TQ], F32, tag="s2")
                    r0 = 2 * kp - R * j
                    off0 = 128 * r0 if (j > 0 and r0 >= 1) else 0
                    for half in range(2):
                        k = 2 * kp + half
                        r = k - R * j
                        off = 128 * r if (j > 0 and r >= 1) else 0
                        nc.tensor.matmul(
                            s2[:, half * TQ + off:(half + 1) * TQ],
                            qkT[:, 1, h, k * 128:(k + 1) * 128],
                            qkT[:, 0, h, j * TQ + off:(j + 1) * TQ],
                            start=True,
                            stop=True,
                        )
                    p2 = pP.tile([128, 2 * TQ], BF16, tag="p2")
                    if off0:
                        nc.scalar.activation(
                            p2.rearrange("p (k t) -> p k t", k=2)[:, :, off0:],
                            s2.rearrange("p (k t) -> p k t", k=2)[:, :, off0:],
                            AF.Exp, scale=sm_scale)
                    else:
                        nc.scalar.activation(p2, s2, AF.Exp, scale=sm_scale)
                    if kp >= npairs - 2:  # the two diagonal-block pairs
                        r0m = 2 * kp - R * j
                        nc.vector.tensor_mul(
                            p2, p2, masks_t[:, r0m * TQ:(r0m + 2) * TQ]
                        )
                    if kp == 0:
                        nc.vector.tensor_copy(daccA, p2)
                    elif kp == 1:
                        nc.gpsimd.dma_start(daccB, p2)
                    elif kp % 2 == 0:
                        nc.vector.tensor_add(daccA, daccA, p2)
                    else:
                        nc.gpsimd.dma_start(daccB, p2, accum_op=ALU.add)
                    plist.append(p2)
                # fold the A-halves now so the add clears DVE before the next
                # head's backlog (consumed by emit_den one head later)
                dsum = dP.tile([128, TQ], BF16, tag="dsum")
                nc.vector.tensor_add(dsum, daccA[:, :TQ], daccA[:, TQ:])
                yps = ps_yp.tile([128, TQ], F32, tag="yps")
                for k in range(nk):
                    r = k - R * j
                    off = 128 * r if r >= 1 else 0
                    nc.tensor.matmul(
                        yps[:, off:],
                        v_all[:, k, h * hd:(h + 1) * hd],
                        plist[k // 2][:, (k % 2) * TQ + off:(k % 2 + 1) * TQ],
                        start=(k == 0),
                        stop=(k == nk - 1),
                    )
                # unnormalized y off PSUM right away (frees the bank; lets
                # the denominator pipeline run two heads deep)
                yb = ybP.tile([128, TQ], BF16, tag="yb")
                if h % 2 == 0:
                    nc.scalar.copy(yb, yps)
                else:
                    nc.vector.tensor_copy(yb, yps)
                if len(pend) >= 2:
                    emit_den(*pend.pop(0))
                pend.append((j, h, dsum, daccB, yb))
                # interleave previous block's projection between heads
                if j > 0 and h == 1:
                    emit_proj(j - 1, 0, NO // 2)
                elif j > 0 and h == 3:
                    emit_proj(j - 1, NO // 2, NO)
        while pend:
            emit_den(*pend.pop(0))
        emit_proj(NJ - 1, 0, NO)

    nc.compile()
    return nc


def make_host_inputs(x, Wqkv, Wproj, q_norm_w, k_norm_w, rope_cos, rope_sin,
                     T_=T, C_=C, HG_=HG, hd=HD, TQ=512):
    """Build the 8 per-core input maps (sharding done on host)."""
    H_ = Wqkv.shape[0] // (3 * hd)
    tpw = H_ // HG_
    R = TQ // 128
    HB = hd // 2

    def rope_tables(w):
        # out1 = qb1*(cos*w1) - qb2*(sin*w2); out2 = qb2*(cos*w2) + qb1*(sin*w1)
        w1, w2 = w[:HB], w[HB:]
        A = rope_cos * w1[None, :]
        Bt = rope_sin * w2[None, :]
        Ct = rope_cos * w2[None, :]
        D = rope_sin * w1[None, :]
        return np.ascontiguousarray(
            np.concatenate([A, Bt, Ct, D], axis=1).astype(ml_dtypes.bfloat16)
        )

    rope_q_h = rope_tables(np.asarray(q_norm_w, dtype=np.float32))
    rope_k_h = rope_tables(np.asarray(k_norm_w, dtype=np.float32))

    # diagonal causal masks: pattern r: valid when tk + 128*r <= tq
    tk = np.arange(128)[:, None]
    tq = np.arange(TQ)[None, :]
    masks = np.concatenate(
        [(tk + 128 * r <= tq) for r in range(R)], axis=0
    ).astype(ml_dtypes.bfloat16)

    Wqkv = np.asarray(Wqkv, dtype=np.float32)
    Wproj = np.asarray(Wproj, dtype=np.float32)
    x = np.asarray(x, dtype=np.float32)
    xTs = [
        np.ascontiguousarray(x[b].T).astype(ml_dtypes.bfloat16)
        for b in range(x.shape[0])
    ]

    in_maps = []
    for core in range(N_CORES):
        b = core // tpw
        g = core % tpw
        rs = slice(g * HG_ * hd, (g + 1) * HG_ * hd)
        W_shard = np.concatenate(
            [Wqkv[0 * H_ * hd:][rs.start:rs.stop],
             Wqkv[1 * H_ * hd:][rs.start:rs.stop],
             Wqkv[2 * H_ * hd:][rs.start:rs.stop]], axis=0
        )  # [3*F1, C]
        in_maps.append({
            "ident": np.eye(128, dtype=ml_dtypes.bfloat16),
            "xT": xTs[b],
            "wqkvT": np.ascontiguousarray(W_shard.T).astype(ml_dtypes.bfloat16),
            "wprojT": np.ascontiguousarray(Wproj[:, rs].T).astype(ml_dtypes.bfloat16),
            "rope_q": rope_q_h,
            "rope_k": rope_k_h,
            "masks": masks,
        })
    return in_maps


_NC_CACHE = {}


def run_spmd(inputs, **run_kwargs):
    from concourse.bass_utils import run_bass_kernel_spmd

    x = np.asarray(inputs["x"])
    in_maps = make_host_inputs(
        x, inputs["Wqkv"], inputs["Wproj"], inputs["q_norm_w"],
        inputs["k_norm_w"], inputs["rope_cos"], inputs["rope_sin"],
    )
    if "nc" not in _NC_CACHE:
        _NC_CACHE["nc"] = build_nc()
    nc = _NC_CACHE["nc"]
    res = run_bass_kernel_spmd(nc, in_maps, core_ids=list(range(N_CORES)),
                               **run_kwargs)
    tpw = N_CORES // B
    out = np.zeros((B, T, C), dtype=np.float32)
    for core in range(N_CORES):
        b = core // tpw
        out[b] += res.results[core]["outT"].astype(np.float32).T
    return out, res


def kernel(**inputs):
    out, _ = run_spmd(inputs)
    return out
